# revision 23
# baseline (speedup 1.0000x reference)
"""Trainium2 Bass kernel for nn_HamiltonianVersorNN.

Math: the reference energy reads only blade-0 of the final layer, and the
versor gate h*sigmoid(h[...,0:1]) makes blade-0 evolve as elementwise SiLU.
Backprop therefore collapses exactly to a 2-layer SiLU MLP on blade-0:

    z1 = A x + c1            A  = W1 @ W_in[:, ::32].T          [32, 6]
    z2 = W2 silu(z1) + c2    c1 = W1 @ b_in[::32] + b1[:, 0]
    dx = A.T (W2.T (w3 * silu'(z2)) * silu'(z1))
    out = x + dt * [dx[3:6], -dx[0:3]]

Performance structure (vs the fp32 block-diag baseline):
  * bf16 matmuls (1 cycle/row vs fp32's 4; FWL fast weight loads).
  * silu(z1) = z1*(1+tanh(z1/2))/2 is built by one fused DVE
    scalar_tensor_tensor (w1 = (t1+1)*z1), so layer 2 is a single
    matmul z2 = 0.5*W2 w1 with its bias applied via the per-partition
    ACT bias operand of silu'(z2). ACT: tanh + 2x silu' per tile.
  * biases ride a host-provided ones-row on the x tile (row 24), so no
    rank-1 bias matmuls and no ACT bias chains.
  * the last matmul runs bf16 so its PSUM output can col-tile: chunk
    parity p lands at partitions 32p+{0..23} of one [56, 512] bank,
    halving PSUM readout cost. The kernel emits only the correction
    dt*J*dx; the x passthrough is added host-side in _unshard_out.

Sharding: pure data parallel over B*S*N positions, 8 cores, 16384
positions/core; partition 32*tl + c holds channel c of token 4g+tl.
(fp22 numerics verified on CPU: rel err 4.7e-5 vs jax reference.)
"""

import sys

import numpy as np

if "/opt/trn_rl_repo" not in sys.path:
    sys.path.insert(0, "/opt/trn_rl_repo")

import concourse.bass as bass
import concourse.tile as tile
from concourse import mybir

AF = mybir.ActivationFunctionType
F32 = mybir.dt.float32
F32R = mybir.dt.float32r

N_CORES = 8
B, S, N, D = 32, 256, 16, 6
HIDDEN = 32
BLADES = 32
DT = 0.01

TOK_TOTAL = B * S * N            # 131072 positions
TOK_CORE = TOK_TOTAL // N_CORES  # 16384
TPC = 4                          # tokens packed per 128-partition column
GROUPS = TOK_CORE // TPC         # 4096 columns per core
FD = 512                         # free-dim per tile (1 PSUM bank fp32)
N_TILES = GROUPS // FD           # 8

KP = TPC * D                     # 24 x/out partitions per lane block
XROWS = KP + 1                   # + ones row for bias folding
ORROWS = 56                      # folded out rows: 32p + (0..23), p in {0,1}
SFD = 2 * FD                     # 1024-column super-tile (ACT/DVE op span)
BF16 = mybir.dt.bfloat16
OGCOLS = GROUPS // 2             # 2048 output columns (2 chunks/bank)


def _build_nc():
    nc = bass.Bass()

    xg = nc.dram_tensor("xg", [XROWS, GROUPS], BF16, kind="ExternalInput")
    # all stationaries packed into one load: cols [l2b | l3 | l4 | l1]
    wp = nc.dram_tensor("wp", [128, 536], BF16, kind="ExternalInput")
    cb = nc.dram_tensor("cb", [128, 1], F32, kind="ExternalInput")
    og = nc.dram_tensor("og", [ORROWS, OGCOLS], F32, kind="ExternalOutput")

    NSUP = N_TILES // 2

    with tile.TileContext(nc) as tc:
        with (
            tc.tile_pool(name="consts", bufs=1) as consts,
            tc.tile_pool(name="xin", bufs=1) as xin,
            tc.tile_pool(name="work", bufs=3) as work,
            tc.tile_pool(name="outp", bufs=3) as outp,
            tc.tile_pool(name="ps1", bufs=2, space="PSUM") as ps1,
            tc.tile_pool(name="ps2", bufs=1, space="PSUM") as ps2,
            tc.tile_pool(name="psv", bufs=1, space="PSUM") as psv,
        ):
            # First x group first so mm1 of super 0 can start ASAP; the
            # remaining loads trickle in behind it on the same queue.
            xw = [xin.tile([XROWS, SFD], BF16, name=f"xw{g}") for g in range(NSUP)]
            sb_wp = consts.tile([128, 536], BF16)
            nc.gpsimd.dma_start(out=sb_wp[:], in_=wp[:])
            sb_l2b = sb_wp[:, 0:128]
            sb_l3 = sb_wp[:, 128:256]
            sb_l4 = sb_wp[:, 256 : 256 + KP]
            sb_l1 = sb_wp[0:XROWS, 280:408]
            # x halves on the sync queue, so mm1 of super 0 is gated only by
            # the first 98 KB transfer.
            nc.sync.dma_start(out=xw[0][:], in_=xg[:, bass.ts(0, SFD)])
            nc.sync.dma_start(out=xw[1][:], in_=xg[:, bass.ts(1, SFD)])
            sb_cb = consts.tile([128, 1], F32)
            nc.gpsimd.dma_start(out=sb_cb[:], in_=cb[:])
            nc.sync.dma_start(out=xw[2][:], in_=xg[:, bass.ts(2, SFD)])
            nc.sync.dma_start(out=xw[3][:], in_=xg[:, bass.ts(3, SFD)])



            # PE warmer: the HAM activity monitor opens the PE clock gate
            # (1.2 -> 2.4 GHz) only after ~3.4us of dense array activity, so
            # run dense K=128 dummies through the DMA window and keep a few
            # "bridge" dummies in the pipeline-fill gaps that would otherwise
            # re-throttle the gate (>3.4us PE idle).
            wmov = consts.tile([128, FD], BF16)
            nc.vector.memset(wmov[:], 0.25)
            wst = consts.tile([128, 128], BF16)
            nc.vector.memset(wst[:], 0.25)
            zdum = ps2.tile([128, FD], F32, tag="z2", name="zdum")

            def warm_pe(n):
                for _ in range(n):
                    nc.tensor.matmul(zdum[:], wst[:], wmov[:], start=True, stop=True)

            warm_pe(12)

            # Dummy first activation: walrus attaches the ACT table load to
            # the first Activation instruction, which can then carry only a
            # single sync wait. Give it a single-wait warm-up op.
            warm_in = consts.tile([1, 128], F32)
            nc.vector.memset(warm_in[:], 0.25)
            warm = consts.tile([1, 128], F32)
            nc.scalar.activation(warm[:], warm_in[:], AF.Derivative_silu)

            # Software pipeline, one-super skew, hand-interleaved so every
            # engine queue alternates ready back-half work of super s-1 with
            # front-half work of super s (in-order queues head-block
            # otherwise). PSUM: z1 2x2 banks, z2 1x2, v1/po share one 2-bank
            # slot (their lifetimes are sequential by construction).
            st = {}

            # Super-tile schedule in FD(=512)-column units: three 1024-wide
            # supers, then two 512-wide ones so the pipeline drain at the end
            # is a short chain. (cu, w): start column-unit and width.
            SUPS = [(0, 2), (2, 2), (4, 2), (6, 1), (7, 1)]
            NS = len(SUPS)

            for s in range(NS + 1):
                live_f = s < NS       # emit front of super s
                live_b = s >= 1       # emit back of super s-1
                if live_f:
                    cu, w = SUPS[s]
                    wid = w * FD
                    xsl = [
                        xw[(cu + p) // 2][:, bass.ts((cu + p) % 2, FD)]
                        for p in range(w)
                    ]

                if live_b:
                    bb = st.pop(s - 1)
                    # d2 = silu'(z2 + c2) (bias via the per-partition ACT arg)
                    d2 = work.tile([128, bb["wid"]], BF16, tag="d2", name=f"d2_{s-1}")
                    nc.scalar.activation(
                        d2[:], bb["z2"][:], AF.Derivative_silu, bias=sb_cb[:]
                    )

                if live_f:
                    # z1 = blockdiag(A) @ x + c1 (bias via ones row)
                    z1 = ps1.tile([128, wid], F32, tag="z1", name=f"z1_{s}")
                    for p in range(w):
                        nc.tensor.matmul(
                            z1[:, bass.ts(p, FD)],
                            sb_l1[:],
                            xsl[p],
                            start=True,
                            stop=True,
                        )
                    # t1 = tanh(z1/2)
                    t1 = work.tile([128, wid], F32, tag="t1", name=f"t1_{s}")
                    nc.scalar.activation(t1[:], z1[:], AF.Tanh, scale=0.5)
                    if s <= 1:
                        warm_pe(3)

                if live_b:
                    # v1 = blockdiag(w3*W2)^T-contraction @ d2
                    v1 = psv.tile([128, bb["wid"]], F32, tag="vpo", name=f"v1_{s-1}")
                    for p in range(bb["w"]):
                        nc.tensor.matmul(
                            v1[:, bass.ts(p, FD)],
                            sb_l3[:],
                            d2[:, bass.ts(p, FD)],
                            start=True,
                            stop=True,
                        )
                    # g1 = v1 * d1 (bf16 feeds the col-tiled bf16 mm4)
                    g1 = work.tile([128, bb["wid"]], BF16, tag="g1", name=f"g1_{s-1}")
                    nc.vector.tensor_mul(g1[:], v1[:], bb["d1"][:])

                if live_f:
                    # d1 = silu'(z1)
                    d1 = work.tile([128, wid], F32, tag="d1", name=f"d1_{s}")
                    nc.scalar.activation(d1[:], z1[:], AF.Derivative_silu)

                if live_b:
                    # po = dt*J*(A^T g1); 1024-supers parity-fold the two
                    # halves at partitions {0,32}+(0..23) of one bank
                    porows = ORROWS if bb["w"] == 2 else KP
                    po = psv.tile([porows, FD], F32, tag="vpo", name=f"po_{s-1}")
                    for p in range(bb["w"]):
                        pr = 32 * p if bb["w"] == 2 else 0
                        nc.tensor.matmul(
                            po[pr : pr + KP, :],
                            sb_l4[:],
                            g1[:, bass.ts(p, FD)],
                            start=True,
                            stop=True,
                        )

                if live_f:
                    # w1 = z1 * (1 + tanh(z1/2)) = 2*silu(z1), one fused op
                    w1 = work.tile([128, wid], BF16, tag="w1", name=f"w1_{s}")
                    nc.vector.scalar_tensor_tensor(
                        w1[:],
                        t1[:],
                        1.0,
                        z1[:],
                        mybir.AluOpType.add,
                        mybir.AluOpType.mult,
                    )

                    # z2 = 0.5*W2 w1 (+ c2 later via the ACT bias of d2)
                    z2 = ps2.tile([128, wid], F32, tag="z2", name=f"z2_{s}")
                    for p in range(w):
                        nc.tensor.matmul(
                            z2[:, bass.ts(p, FD)],
                            sb_l2b[:],
                            w1[:, bass.ts(p, FD)],
                            start=True,
                            stop=True,
                        )
                    st[s] = {"z2": z2, "d1": d1, "w": w, "wid": wid, "cu": cu}

                if live_b:
                    # PSUM readout; og holds the dt*J*dx correction only (the
                    # x passthrough is added host-side during unshard)
                    cu_b, w_b = bb["cu"], bb["w"]
                    ocs = bass.ts(cu_b // 2, FD)
                    osb = outp.tile([ORROWS, FD], F32, tag="osb", name=f"osb_{s-1}")
                    if w_b == 2:
                        nc.vector.tensor_copy(osb[:], po[:])
                        nc.sync.dma_start(out=og[:, ocs], in_=osb[:])
                    else:
                        pr = cu_b % 2
                        orr = slice(32 * pr, 32 * pr + KP)
                        nc.vector.tensor_copy(osb[0:KP, :], po[:])
                        nc.sync.dma_start(out=og[orr, ocs], in_=osb[0:KP, :])

    return nc


def _split_multi_waits(nc):
    """This walrus build rejects engine instructions carrying more than one
    sync wait ("Too many sync wait commands"). Hoist all but one wait of
    each instruction onto standalone NoOps issued just before it on the
    same engine (engines execute their queue in order, so semantics are
    preserved)."""
    for f in nc.m.functions:
        for b in f.blocks:
            insts = list(b.instructions)
            out = []
            changed = False
            for inst in insts:
                # This walrus build also rejects the raw-ISA
                # EVENT_SEMAPHORE_RANGE_CLEAR Tile emits at context end
                # ("ISA wrong length" — ISA table version skew). The NEFF
                # preamble re-initializes semaphores, so drop it.
                if (
                    type(inst).__name__ == "InstISA"
                    and getattr(inst, "op_name", "") == "EVENT_SEMAPHORE_RANGE_CLEAR"
                ):
                    changed = True
                    continue
                si = getattr(inst, "sync_info", None)
                waits = list(si.on_wait) if si is not None and si.on_wait else []
                if len(waits) > 1:
                    changed = True
                    for k, w in enumerate(waits[:-1]):
                        nop = mybir.InstNoOp(name=f"{inst.name}-w{k}", ins=[], outs=[])
                        nop.engine = inst.engine
                        nop.sync_info = mybir.SyncInfo(on_wait=[w], on_update=[])
                        out.append(nop)
                    inst.sync_info = mybir.SyncInfo(
                        on_wait=[waits[-1]], on_update=list(si.on_update or [])
                    )
                out.append(inst)
            if changed:
                b.instructions = out
    return nc


_NC_CACHE = None


def _get_nc():
    global _NC_CACHE
    if _NC_CACHE is None:
        _NC_CACHE = _split_multi_waits(_build_nc())
    return _NC_CACHE


def _prep_weights(W_in, b_in, W1, b1, W2, b2, W3, b3):
    """Host-side constant folding into the kernel's stationary layouts."""
    W_in = np.asarray(W_in, np.float64)
    b_in = np.asarray(b_in, np.float64)
    W1 = np.asarray(W1, np.float64)
    b1 = np.asarray(b1, np.float64)
    W2 = np.asarray(W2, np.float64)
    b2 = np.asarray(b2, np.float64)
    W3 = np.asarray(W3, np.float64)

    Win0 = W_in[:, ::BLADES]            # [6, 8]
    bin0 = b_in[::BLADES]               # [8]
    A = W1 @ Win0.T                     # [32, 6]
    c1 = W1 @ bin0 + b1[:, 0]           # [32]
    c2 = b2[:, 0]                       # [32]
    w3 = W3[0, :]                       # [32]

    W2A = 0.5 * (W2 @ A)                # [32, 6]
    cz2 = 0.5 * (W2 @ c1) + c2          # [32]

    # Bout[d, c]: out[d] += dt*dx[d+3] (d<3), -dt*dx[d-3] (d>=3); dx = A^T g1
    Bout = np.zeros((D, HIDDEN))
    Bout[0:3, :] = DT * A[:, 3:6].T
    Bout[3:6, :] = -DT * A[:, 0:3].T

    import ml_dtypes

    bf = ml_dtypes.bfloat16
    l1 = np.zeros((XROWS, 128), bf)
    l2a = np.zeros((XROWS, 128), bf)
    l2b = np.zeros((128, 128), bf)
    l3 = np.zeros((128, 128), bf)
    l4 = np.zeros((128, KP), bf)
    for tl in range(TPC):
        r0, c0 = 6 * tl, 32 * tl
        # z1[32tl+c] = sum_d A[c,d] x[d] + c1[c]
        l1[r0 : r0 + 6, c0 : c0 + 32] = A.T.astype(bf)
        l1[KP, c0 : c0 + 32] = c1.astype(bf)
        # z2 partial from x: 0.5*(W2A)[j,d] + bias row
        l2a[r0 : r0 + 6, c0 : c0 + 32] = W2A.T.astype(bf)
        l2a[KP, c0 : c0 + 32] = cz2.astype(bf)
        # z2 partial from u1: 0.5*W2[j,i]
        l2b[c0 : c0 + 32, c0 : c0 + 32] = (0.5 * W2.T).astype(bf)
        # v1[j] = sum_i w3[i] W2[i,j] d2[i]
        l3[c0 : c0 + 32, c0 : c0 + 32] = (w3[:, None] * W2).astype(bf)
        # out[6tl+d] += Bout[d, c] g1[32tl+c]
        l4[c0 : c0 + 32, r0 : r0 + 6] = Bout.T.astype(bf)

    wpk = np.zeros((128, 536), bf)
    wpk[:, 0:128] = l2b
    wpk[:, 128:256] = l3
    wpk[:, 256 : 256 + KP] = l4
    wpk[:XROWS, 280:408] = l1
    cbv = np.zeros((128, 1), np.float32)
    for tl in range(TPC):
        cbv[32 * tl : 32 * tl + 32, 0] = c2.astype(np.float32)
    return {"wp": wpk, "cb": cbv}


def _shard_x(x):
    """[B,S,N,D] -> per-core [25, GROUPS] lane layout (+ ones row)."""
    import ml_dtypes

    xf = np.ascontiguousarray(np.asarray(x, np.float32)).reshape(TOK_TOTAL, D)
    shards = []
    for c in range(N_CORES):
        xc = xf[c * TOK_CORE : (c + 1) * TOK_CORE]          # [16384, 6]
        lane = xc.reshape(GROUPS, TPC, D).transpose(1, 2, 0).reshape(KP, GROUPS)
        xgc = np.empty((XROWS, GROUPS), ml_dtypes.bfloat16)
        xgc[:KP] = lane.astype(ml_dtypes.bfloat16)
        xgc[KP] = 1.0
        shards.append(xgc)
    return shards


def _unshard_out(x, outs):
    """per-core [56, 2048] folded corrections + x -> [B,S,N,D]."""
    full = np.empty((TOK_TOTAL, D), np.float32)
    for c, ogc in enumerate(outs):
        ogc = np.asarray(ogc)
        # row 32p + 6tl + d, col 512s + cc -> token 4*(512*(2s+p)+cc)+tl
        o = np.stack([ogc[0:KP], ogc[32 : 32 + KP]])        # [p, 24, 2048]
        o = o.reshape(2, TPC, D, N_TILES // 2, FD)          # [p, tl, d, s, cc]
        o = o.transpose(3, 0, 4, 1, 2).reshape(TOK_CORE, D)  # [s, p, cc, tl, d]
        full[c * TOK_CORE : (c + 1) * TOK_CORE] = o
    out = full.reshape(B, S, N, D)
    out += np.asarray(x, np.float32)
    return out


# Test-harness knobs (ignored in normal use): set kernel._TRACE = True to
# collect an NTFF profile; the BassKernelResults lands in kernel._LAST_RES.
_TRACE = False
_LAST_RES = None


def kernel(x, W_in, b_in, W1, b1, W2, b2, W3, b3):
    global _LAST_RES
    from concourse.bass_utils import run_bass_kernel_spmd

    nc = _get_nc()
    consts = _prep_weights(W_in, b_in, W1, b1, W2, b2, W3, b3)
    shards = _shard_x(x)
    in_maps = [{"xg": shards[c], **consts} for c in range(N_CORES)]
    res = run_bass_kernel_spmd(nc, in_maps, list(range(N_CORES)), trace=_TRACE)
    _LAST_RES = res
    return _unshard_out(x, [res.results[c]["og"] for c in range(N_CORES)])


# revision 25
# speedup vs baseline: 1.2175x; 1.2175x over previous
"""Trainium2 Bass kernel for nn_HamiltonianVersorNN.

Math: the reference energy reads only blade-0 of the final layer, and the
versor gate h*sigmoid(h[...,0:1]) makes blade-0 evolve as elementwise SiLU.
Backprop therefore collapses exactly to a 2-layer SiLU MLP on blade-0:

    z1 = A x + c1            A  = W1 @ W_in[:, ::32].T          [32, 6]
    z2 = W2 silu(z1) + c2    c1 = W1 @ b_in[::32] + b1[:, 0]
    dx = A.T (W2.T (w3 * silu'(z2)) * silu'(z1))
    out = x + dt * [dx[3:6], -dx[0:3]]

Performance structure (vs the fp32 block-diag baseline):
  * bf16 matmuls (1 cycle/row vs fp32's 4; FWL fast weight loads).
  * silu(z1) = z1*(1+tanh(z1/2))/2 is built by one fused DVE
    scalar_tensor_tensor (w1 = (t1+1)*z1), so layer 2 is a single
    matmul z2 = 0.5*W2 w1 with its bias applied via the per-partition
    ACT bias operand of silu'(z2). ACT: tanh + 2x silu' per tile.
  * biases ride a host-provided ones-row on the x tile (row 24), so no
    rank-1 bias matmuls and no ACT bias chains.
  * the last matmul runs bf16 so its PSUM output can col-tile: chunk
    parity p lands at partitions 32p+{0..23} of one [56, 512] bank,
    halving PSUM readout cost. The kernel emits only the correction
    dt*J*dx; the x passthrough is added host-side in _unshard_out.

Sharding: pure data parallel over B*S*N positions, 8 cores, 16384
positions/core; partition 32*tl + c holds channel c of token 4g+tl.
(fp22 numerics verified on CPU: rel err 4.7e-5 vs jax reference.)
"""

import sys

import numpy as np

if "/opt/trn_rl_repo" not in sys.path:
    sys.path.insert(0, "/opt/trn_rl_repo")

import concourse.bass as bass
import concourse.tile as tile
from concourse import mybir

AF = mybir.ActivationFunctionType
F32 = mybir.dt.float32
F32R = mybir.dt.float32r

N_CORES = 8
B, S, N, D = 32, 256, 16, 6
HIDDEN = 32
BLADES = 32
DT = 0.01

TOK_TOTAL = B * S * N            # 131072 positions
TOK_CORE = TOK_TOTAL // N_CORES  # 16384
TPC = 4                          # tokens packed per 128-partition column
GROUPS = TOK_CORE // TPC         # 4096 columns per core
FD = 512                         # free-dim per tile (1 PSUM bank fp32)
N_TILES = GROUPS // FD           # 8

KP = TPC * D                     # 24 x/out partitions per lane block
XROWS = KP + 1                   # + ones row for bias folding
ORROWS = 56                      # folded out rows: 32p + (0..23), p in {0,1}
SFD = 2 * FD                     # 1024-column super-tile (ACT/DVE op span)
BF16 = mybir.dt.bfloat16
OGCOLS = GROUPS // 2             # 2048 output columns (2 chunks/bank)


def _build_nc():
    nc = bass.Bass()

    xg = nc.dram_tensor("xg", [XROWS, GROUPS], BF16, kind="ExternalInput")
    # all stationaries packed into one load: cols [l2b | l3 | l4 | l1]
    wp = nc.dram_tensor("wp", [128, 536], BF16, kind="ExternalInput")
    cb = nc.dram_tensor("cb", [128, 1], F32, kind="ExternalInput")
    og = nc.dram_tensor("og", [ORROWS, OGCOLS], F32, kind="ExternalOutput")

    NSUP = N_TILES // 2

    with tile.TileContext(nc) as tc:
        with (
            tc.tile_pool(name="consts", bufs=1) as consts,
            tc.tile_pool(name="xin", bufs=1) as xin,
            tc.tile_pool(name="work", bufs=3) as work,
            tc.tile_pool(name="outp", bufs=3) as outp,
            tc.tile_pool(name="ps1", bufs=2, space="PSUM") as ps1,
            tc.tile_pool(name="ps2", bufs=1, space="PSUM") as ps2,
            tc.tile_pool(name="psv", bufs=1, space="PSUM") as psv,
        ):
            # First x group first so mm1 of super 0 can start ASAP; the
            # remaining loads trickle in behind it on the same queue.
            xw = [xin.tile([XROWS, SFD], BF16, name=f"xw{g}") for g in range(NSUP)]
            sb_wp = consts.tile([128, 536], BF16)
            nc.gpsimd.dma_start(out=sb_wp[:], in_=wp[:])
            sb_l2b = sb_wp[:, 0:128]
            sb_l3 = sb_wp[:, 128:256]
            sb_l4 = sb_wp[:, 256 : 256 + KP]
            sb_l1 = sb_wp[0:XROWS, 280:408]
            # x halves on the sync queue, so mm1 of super 0 is gated only by
            # the first 98 KB transfer.
            nc.sync.dma_start(out=xw[0][:], in_=xg[:, bass.ts(0, SFD)])
            nc.sync.dma_start(out=xw[1][:], in_=xg[:, bass.ts(1, SFD)])
            sb_cb = consts.tile([128, 1], F32)
            nc.gpsimd.dma_start(out=sb_cb[:], in_=cb[:])
            nc.sync.dma_start(out=xw[2][:], in_=xg[:, bass.ts(2, SFD)])
            nc.sync.dma_start(out=xw[3][:], in_=xg[:, bass.ts(3, SFD)])



            # PE warmer on the vpo bank (v1 is not needed until ~18us, so
            # unlike the z2-tag variants this cannot gate the early pipeline):
            # ~10 dense K=128 dummies open the HAM clock gate (1.2 -> 2.4 GHz)
            # before the real matmuls arrive.
            wmov = consts.tile([128, FD], BF16)
            nc.vector.memset(wmov[:], 0.25)
            wst = consts.tile([128, 128], BF16)
            nc.vector.memset(wst[:], 0.25)
            zdum = psv.tile([128, FD], F32, tag="vpo", name="zdum")
            for _ in range(10):
                nc.tensor.matmul(zdum[:], wst[:], wmov[:], start=True, stop=True)

            # Dummy first activation: walrus attaches the ACT table load to
            # the first Activation instruction, which can then carry only a
            # single sync wait. Give it a single-wait warm-up op.
            warm_in = consts.tile([1, 128], F32)
            nc.vector.memset(warm_in[:], 0.25)
            warm = consts.tile([1, 128], F32)
            nc.scalar.activation(warm[:], warm_in[:], AF.Derivative_silu)

            # Software pipeline, one-super skew, hand-interleaved so every
            # engine queue alternates ready back-half work of super s-1 with
            # front-half work of super s (in-order queues head-block
            # otherwise). PSUM: z1 2x2 banks, z2 1x2, v1/po share one 2-bank
            # slot (their lifetimes are sequential by construction).
            st = {}

            # Super-tile schedule in FD(=512)-column units: three 1024-wide
            # supers, then two 512-wide ones so the pipeline drain at the end
            # is a short chain. (cu, w): start column-unit and width.
            SUPS = [(0, 2), (2, 2), (4, 2), (6, 1), (7, 1)]
            NS = len(SUPS)

            for s in range(NS + 1):
                live_f = s < NS       # emit front of super s
                live_b = s >= 1       # emit back of super s-1
                if live_f:
                    cu, w = SUPS[s]
                    wid = w * FD
                    xsl = [
                        xw[(cu + p) // 2][:, bass.ts((cu + p) % 2, FD)]
                        for p in range(w)
                    ]

                if live_b:
                    bb = st.pop(s - 1)
                    # d2 = silu'(z2 + c2) (bias via the per-partition ACT arg)
                    d2 = work.tile([128, bb["wid"]], BF16, tag="d2", name=f"d2_{s-1}")
                    nc.scalar.activation(
                        d2[:], bb["z2"][:], AF.Derivative_silu, bias=sb_cb[:]
                    )

                if live_f:
                    # z1 = blockdiag(A) @ x + c1 (bias via ones row)
                    z1 = ps1.tile([128, wid], F32, tag="z1", name=f"z1_{s}")
                    for p in range(w):
                        nc.tensor.matmul(
                            z1[:, bass.ts(p, FD)],
                            sb_l1[:],
                            xsl[p],
                            start=True,
                            stop=True,
                        )
                    # t1 = tanh(z1/2)
                    t1 = work.tile([128, wid], F32, tag="t1", name=f"t1_{s}")
                    nc.scalar.activation(t1[:], z1[:], AF.Tanh, scale=0.5)

                if live_b:
                    # v1 = blockdiag(w3*W2)^T-contraction @ d2
                    v1 = psv.tile([128, bb["wid"]], F32, tag="vpo", name=f"v1_{s-1}")
                    for p in range(bb["w"]):
                        nc.tensor.matmul(
                            v1[:, bass.ts(p, FD)],
                            sb_l3[:],
                            d2[:, bass.ts(p, FD)],
                            start=True,
                            stop=True,
                        )
                    # g1 = v1 * d1 (bf16 feeds the col-tiled bf16 mm4)
                    g1 = work.tile([128, bb["wid"]], BF16, tag="g1", name=f"g1_{s-1}")
                    nc.vector.tensor_mul(g1[:], v1[:], bb["d1"][:])

                if live_f:
                    # d1 = silu'(z1)
                    d1 = work.tile([128, wid], F32, tag="d1", name=f"d1_{s}")
                    nc.scalar.activation(d1[:], z1[:], AF.Derivative_silu)

                if live_b:
                    # po = dt*J*(A^T g1); 1024-supers parity-fold the two
                    # halves at partitions {0,32}+(0..23) of one bank
                    porows = ORROWS if bb["w"] == 2 else KP
                    po = psv.tile([porows, FD], F32, tag="vpo", name=f"po_{s-1}")
                    for p in range(bb["w"]):
                        pr = 32 * p if bb["w"] == 2 else 0
                        nc.tensor.matmul(
                            po[pr : pr + KP, :],
                            sb_l4[:],
                            g1[:, bass.ts(p, FD)],
                            start=True,
                            stop=True,
                        )

                if live_f:
                    # w1 = z1 * (1 + tanh(z1/2)) = 2*silu(z1), one fused op
                    w1 = work.tile([128, wid], BF16, tag="w1", name=f"w1_{s}")
                    nc.vector.scalar_tensor_tensor(
                        w1[:],
                        t1[:],
                        1.0,
                        z1[:],
                        mybir.AluOpType.add,
                        mybir.AluOpType.mult,
                    )

                    # z2 = 0.5*W2 w1 (+ c2 later via the ACT bias of d2)
                    z2 = ps2.tile([128, wid], F32, tag="z2", name=f"z2_{s}")
                    for p in range(w):
                        nc.tensor.matmul(
                            z2[:, bass.ts(p, FD)],
                            sb_l2b[:],
                            w1[:, bass.ts(p, FD)],
                            start=True,
                            stop=True,
                        )
                    st[s] = {"z2": z2, "d1": d1, "w": w, "wid": wid, "cu": cu}

                if live_b:
                    # PSUM readout; og holds the dt*J*dx correction only (the
                    # x passthrough is added host-side during unshard)
                    cu_b, w_b = bb["cu"], bb["w"]
                    ocs = bass.ts(cu_b // 2, FD)
                    osb = outp.tile([ORROWS, FD], F32, tag="osb", name=f"osb_{s-1}")
                    if w_b == 2:
                        nc.vector.tensor_copy(osb[:], po[:])
                        nc.sync.dma_start(out=og[:, ocs], in_=osb[:])
                    else:
                        pr = cu_b % 2
                        orr = slice(32 * pr, 32 * pr + KP)
                        nc.vector.tensor_copy(osb[0:KP, :], po[:])
                        nc.sync.dma_start(out=og[orr, ocs], in_=osb[0:KP, :])

    return nc


def _split_multi_waits(nc):
    """This walrus build rejects engine instructions carrying more than one
    sync wait ("Too many sync wait commands"). Hoist all but one wait of
    each instruction onto standalone NoOps issued just before it on the
    same engine (engines execute their queue in order, so semantics are
    preserved)."""
    for f in nc.m.functions:
        for b in f.blocks:
            insts = list(b.instructions)
            out = []
            changed = False
            for inst in insts:
                # This walrus build also rejects the raw-ISA
                # EVENT_SEMAPHORE_RANGE_CLEAR Tile emits at context end
                # ("ISA wrong length" — ISA table version skew). The NEFF
                # preamble re-initializes semaphores, so drop it.
                if (
                    type(inst).__name__ == "InstISA"
                    and getattr(inst, "op_name", "") == "EVENT_SEMAPHORE_RANGE_CLEAR"
                ):
                    changed = True
                    continue
                si = getattr(inst, "sync_info", None)
                waits = list(si.on_wait) if si is not None and si.on_wait else []
                if len(waits) > 1:
                    changed = True
                    for k, w in enumerate(waits[:-1]):
                        nop = mybir.InstNoOp(name=f"{inst.name}-w{k}", ins=[], outs=[])
                        nop.engine = inst.engine
                        nop.sync_info = mybir.SyncInfo(on_wait=[w], on_update=[])
                        out.append(nop)
                    inst.sync_info = mybir.SyncInfo(
                        on_wait=[waits[-1]], on_update=list(si.on_update or [])
                    )
                out.append(inst)
            if changed:
                b.instructions = out
    return nc


_NC_CACHE = None


def _get_nc():
    global _NC_CACHE
    if _NC_CACHE is None:
        _NC_CACHE = _split_multi_waits(_build_nc())
    return _NC_CACHE


def _prep_weights(W_in, b_in, W1, b1, W2, b2, W3, b3):
    """Host-side constant folding into the kernel's stationary layouts."""
    W_in = np.asarray(W_in, np.float64)
    b_in = np.asarray(b_in, np.float64)
    W1 = np.asarray(W1, np.float64)
    b1 = np.asarray(b1, np.float64)
    W2 = np.asarray(W2, np.float64)
    b2 = np.asarray(b2, np.float64)
    W3 = np.asarray(W3, np.float64)

    Win0 = W_in[:, ::BLADES]            # [6, 8]
    bin0 = b_in[::BLADES]               # [8]
    A = W1 @ Win0.T                     # [32, 6]
    c1 = W1 @ bin0 + b1[:, 0]           # [32]
    c2 = b2[:, 0]                       # [32]
    w3 = W3[0, :]                       # [32]

    W2A = 0.5 * (W2 @ A)                # [32, 6]
    cz2 = 0.5 * (W2 @ c1) + c2          # [32]

    # Bout[d, c]: out[d] += dt*dx[d+3] (d<3), -dt*dx[d-3] (d>=3); dx = A^T g1
    Bout = np.zeros((D, HIDDEN))
    Bout[0:3, :] = DT * A[:, 3:6].T
    Bout[3:6, :] = -DT * A[:, 0:3].T

    import ml_dtypes

    bf = ml_dtypes.bfloat16
    l1 = np.zeros((XROWS, 128), bf)
    l2a = np.zeros((XROWS, 128), bf)
    l2b = np.zeros((128, 128), bf)
    l3 = np.zeros((128, 128), bf)
    l4 = np.zeros((128, KP), bf)
    for tl in range(TPC):
        r0, c0 = 6 * tl, 32 * tl
        # z1[32tl+c] = sum_d A[c,d] x[d] + c1[c]
        l1[r0 : r0 + 6, c0 : c0 + 32] = A.T.astype(bf)
        l1[KP, c0 : c0 + 32] = c1.astype(bf)
        # z2 partial from x: 0.5*(W2A)[j,d] + bias row
        l2a[r0 : r0 + 6, c0 : c0 + 32] = W2A.T.astype(bf)
        l2a[KP, c0 : c0 + 32] = cz2.astype(bf)
        # z2 partial from u1: 0.5*W2[j,i]
        l2b[c0 : c0 + 32, c0 : c0 + 32] = (0.5 * W2.T).astype(bf)
        # v1[j] = sum_i w3[i] W2[i,j] d2[i]
        l3[c0 : c0 + 32, c0 : c0 + 32] = (w3[:, None] * W2).astype(bf)
        # out[6tl+d] += Bout[d, c] g1[32tl+c]
        l4[c0 : c0 + 32, r0 : r0 + 6] = Bout.T.astype(bf)

    wpk = np.zeros((128, 536), bf)
    wpk[:, 0:128] = l2b
    wpk[:, 128:256] = l3
    wpk[:, 256 : 256 + KP] = l4
    wpk[:XROWS, 280:408] = l1
    cbv = np.zeros((128, 1), np.float32)
    for tl in range(TPC):
        cbv[32 * tl : 32 * tl + 32, 0] = c2.astype(np.float32)
    return {"wp": wpk, "cb": cbv}


def _shard_x(x):
    """[B,S,N,D] -> per-core [25, GROUPS] lane layout (+ ones row)."""
    import ml_dtypes

    xf = np.ascontiguousarray(np.asarray(x, np.float32)).reshape(TOK_TOTAL, D)
    shards = []
    for c in range(N_CORES):
        xc = xf[c * TOK_CORE : (c + 1) * TOK_CORE]          # [16384, 6]
        lane = xc.reshape(GROUPS, TPC, D).transpose(1, 2, 0).reshape(KP, GROUPS)
        xgc = np.empty((XROWS, GROUPS), ml_dtypes.bfloat16)
        xgc[:KP] = lane.astype(ml_dtypes.bfloat16)
        xgc[KP] = 1.0
        shards.append(xgc)
    return shards


def _unshard_out(x, outs):
    """per-core [56, 2048] folded corrections + x -> [B,S,N,D]."""
    full = np.empty((TOK_TOTAL, D), np.float32)
    for c, ogc in enumerate(outs):
        ogc = np.asarray(ogc)
        # row 32p + 6tl + d, col 512s + cc -> token 4*(512*(2s+p)+cc)+tl
        o = np.stack([ogc[0:KP], ogc[32 : 32 + KP]])        # [p, 24, 2048]
        o = o.reshape(2, TPC, D, N_TILES // 2, FD)          # [p, tl, d, s, cc]
        o = o.transpose(3, 0, 4, 1, 2).reshape(TOK_CORE, D)  # [s, p, cc, tl, d]
        full[c * TOK_CORE : (c + 1) * TOK_CORE] = o
    out = full.reshape(B, S, N, D)
    out += np.asarray(x, np.float32)
    return out


# Test-harness knobs (ignored in normal use): set kernel._TRACE = True to
# collect an NTFF profile; the BassKernelResults lands in kernel._LAST_RES.
_TRACE = False
_LAST_RES = None


def kernel(x, W_in, b_in, W1, b1, W2, b2, W3, b3):
    global _LAST_RES
    from concourse.bass_utils import run_bass_kernel_spmd

    nc = _get_nc()
    consts = _prep_weights(W_in, b_in, W1, b1, W2, b2, W3, b3)
    shards = _shard_x(x)
    in_maps = [{"xg": shards[c], **consts} for c in range(N_CORES)]
    res = run_bass_kernel_spmd(nc, in_maps, list(range(N_CORES)), trace=_TRACE)
    _LAST_RES = res
    return _unshard_out(x, [res.results[c]["og"] for c in range(N_CORES)])
